# revision 18
# baseline (speedup 1.0000x reference)
"""GAT (2-layer, 8-head) Trainium2 Bass kernel, 8-way node-sharded. v4.

Per-execution input staging costs ~0.7 ms/MB/core on this stack, so inputs
are aggressively minimized (~1.7 MB/core):
  - x is shipped bf16, transposed, per-core shard only.
  - The two big weight matrices (W1|W2 -> [768,512] bf16) are SHARDED over
    cores and AllGathered on-device (98 KB/core instead of 786 KB).
  - The gather index table is shipped as its unique 16 rows and replicated
    to 128 partitions on-device.
  - Pooling one-hot is built on-device from per-node batch-id + 1/cnt
    scalars (10 KB instead of 655 KB).
  - One-hot edge matrices are built ON-CHIP from compact dst_rel arrays
    (drc/drow, 87 KB each).

Compute layout:
  - Host balances nodes across the 160 blocks by in-degree (free node
    permutation) -> tpb drops 19->17 and all cores have identical loads.
  - Feature axis interleaved (c-major, head-fast) so the per-edge attention
    multiply (msg) runs in DVE 2x mode.
  - Per layer: node matmuls (bf16) -> table [feat 512 | a_src 8 | pad],
    AllGather, edge phase: dma_gather per-edge rows; one-hot matmuls on the
    TensorEngine for segment softmax+aggregation (denominators cancel per
    row, normalize once per block).  LeakyReLU / ELU fused via
    scalar_tensor_tensor.  Layer-1 h transposed on-chip via PE transposes
    into SBUF for layer-2 node matmuls.  Mean-pool accumulates in a
    persistent PSUM bank, AllReduce, classifier + log_softmax on-chip.
"""
import os
import sys
import heapq
from contextlib import ExitStack
from dataclasses import dataclass

import numpy as np

sys.path.insert(0, "/opt/trn_rl_repo")

import ml_dtypes  # noqa: E402

import concourse.bass as bass  # noqa: E402
import concourse.tile as tile  # noqa: E402
from concourse import mybir  # noqa: E402
from concourse import library_config  # noqa: E402
from concourse._compat import with_exitstack  # noqa: E402

P = 128
AF = mybir.ActivationFunctionType
ALU = mybir.AluOpType
DT = mybir.dt
BF16 = ml_dtypes.bfloat16

GRP = 4          # pass A/B tile batching (tiles per group)
WBR = 768        # weight blob rows (w1 256 + w2 512)


@dataclass(frozen=True)
class GATConfig:
    n: int = 20000
    e: int = 320000
    in_dim: int = 256
    hid: int = 64
    heads: int = 8
    classes: int = 10
    g: int = 64
    ncore: int = 8
    neg_slope: float = 0.2

    @property
    def d(self):
        return self.hid * self.heads          # 512

    @property
    def nb(self):
        return 20                             # node blocks / core

    @property
    def nloc(self):
        return self.nb * P                    # 2560 padded local rows

    @property
    def tblw(self):
        return self.d + P                     # 640 bf16 -> 1280B rows

    @property
    def ct(self):
        return self.in_dim // P               # contraction tiles layer 1

    @property
    def dt_(self):
        return self.d // P                    # d tiles (4)


CFG = GATConfig()


def _feat_perm(cfg: GATConfig):
    """new feature j=(c,h) <- old feature h*hid+c (head-fast interleave)."""
    j = np.arange(cfg.d)
    return (j % cfg.heads) * cfg.hid + j // cfg.heads


def blob_layout(cfg: GATConfig, tpb: int):
    """Row layout of the single packed bf16 input tensor [R, 512].

    Each entry: name -> (row0, nrows).  All multi-partition views are
    contiguous flat regions reshaped on-device via AP rearrange.
    """
    epb = tpb * P
    nb = cfg.nb
    sizes = [
        ("x_t", cfg.in_dim * cfg.nloc // 2),   # fp8 bits in bf16 container
        ("wsh", (WBR // cfg.ncore) * cfg.d),   # weight-blob shard rows
        ("w1a", cfg.in_dim * 16),
        ("w2a", cfg.d * 16),
        ("b1", cfg.d),
        ("b2", cfg.d),
        ("b1a", 512),                          # 16 used
        ("b2a", 512),
        ("lin_w", cfg.d * 16),                 # width padded 10->16
        ("iota_row", P * P),
        ("iota_col", 512),                     # 128 used
        ("drc", P * nb * tpb),
        ("drow", nb * epb),
        ("g16", 16 * (nb * epb // 16)),        # int16 bits in bf16 container
        ("bid", P * nb * 2),                   # fp32 bits
        ("wcol", P * nb * 2),                  # fp32 bits
        ("lin_b", 512),                        # fp32 bits, 20 bf16 els used
    ]
    out = {}
    r = 0
    for name, els in sizes:
        rows = (els + 511) // 512
        out[name] = (r, rows)
        r += rows
    return out, r


# --------------------------------------------------------------------------
# Host-side preprocessing
# --------------------------------------------------------------------------

def build_host_data(cfg: GATConfig, edge_index: np.ndarray, batch: np.ndarray):
    """Balance nodes over blocks, build per-core compact index arrays."""
    n, ncore, nb, nloc = cfg.n, cfg.ncore, cfg.nb, cfg.nloc
    nblocks = ncore * nb
    src = np.concatenate([edge_index[0].astype(np.int64), np.arange(n, dtype=np.int64)])
    dst = np.concatenate([edge_index[1].astype(np.int64), np.arange(n, dtype=np.int64)])
    deg = np.bincount(dst, minlength=n)

    # greedy degree balancing: heaviest node -> lightest non-full block
    cap = np.full(nblocks, P, dtype=np.int64)
    for c in range(ncore):
        cap[c * nb + nb - 1] = P - 1          # reserve per-core zero row
    order = np.argsort(-deg, kind="stable")
    counts = np.zeros(nblocks, dtype=np.int64)
    totals = np.zeros(nblocks, dtype=np.int64)
    blk_of = np.empty(n, dtype=np.int64)
    slot_in_blk = np.empty(n, dtype=np.int64)
    heap = [(0, b) for b in range(nblocks)]
    heapq.heapify(heap)
    for v in order:
        while True:
            tot, b = heapq.heappop(heap)
            if counts[b] < cap[b]:
                break
        blk_of[v] = b
        slot_in_blk[v] = counts[b]
        counts[b] += 1
        totals[b] += deg[v]
        if counts[b] < cap[b]:
            heapq.heappush(heap, (int(totals[b]), b))

    tpb = int(np.ceil(totals.max() / P))
    epb = tpb * P

    node_core = blk_of // nb
    local_blk = blk_of % nb
    local_slot = local_blk * P + slot_in_blk
    row_id = node_core * nloc + local_slot

    node_at = np.full((ncore, nloc), -1, dtype=np.int64)
    node_at[node_core, local_slot] = np.arange(n)

    cnt_g = np.bincount(batch, minlength=cfg.g).astype(np.float64)
    inv_cnt = (1.0 / np.maximum(cnt_g, 1.0)).astype(np.float32)

    e_core = node_core[dst]
    e_blk = local_blk[dst]
    e_rel = slot_in_blk[dst]

    cores = []
    for c in range(ncore):
        m = e_core == c
        es, eb, er = src[m], e_blk[m], e_rel[m]
        o = np.argsort(eb, kind="stable")
        es, eb, er = es[o], eb[o], er[o]
        cnts = np.bincount(eb, minlength=nb)
        zrow_gid = c * nloc + nloc - 1

        src_tid = np.full((nb, epb), zrow_gid, dtype=np.int64)
        dst_rel = np.full((nb, epb), 255, dtype=np.int64)
        off = 0
        for b in range(nb):
            k = int(cnts[b])
            sl = slice(off, off + k)
            src_tid[b, :k] = row_id[es[sl]]
            dst_rel[b, :k] = er[sl]
            off += k
        assert src_tid.max() < 2 ** 15

        # dma_gather idx, unique 16 rows: [16, nb*epb/16] int16
        g16 = np.zeros((16, nb * epb // 16), dtype=np.int16)
        for b in range(nb):
            g16[:, b * (epb // 16):(b + 1) * (epb // 16)] = \
                src_tid[b].reshape(-1, 16).T.astype(np.int16)

        # dst_rel column form [128, nb*tpb]: A[p, b*tpb+t] = rel[b, t*128+p]
        drc = dst_rel.reshape(nb, tpb, P).transpose(2, 0, 1).reshape(P, nb * tpb)
        drc = np.ascontiguousarray(drc).astype(BF16)
        # dst_rel row form [1, nb*epb] bf16 (broadcast on device via PE)
        drow = dst_rel.reshape(1, nb * epb).astype(BF16)

        # per-slot batch id + prescaled 1/cnt: [128, nb] each
        bid = np.full((P, nb), 255.0, dtype=np.float32)
        wcol = np.zeros((P, nb), dtype=np.float32)
        for b in range(nb):
            nodes = node_at[c, b * P:(b + 1) * P]
            real = nodes >= 0
            bid[real, b] = batch[nodes[real]]
            wcol[real, b] = inv_cnt[batch[nodes[real]]]

        cores.append(dict(g16=g16, drc=drc, drow=drow,
                          bid=bid, wcol=wcol))

    consts = dict(node_at=node_at, tpb=tpb)
    return tpb, cores, consts


def build_weight_data(cfg: GATConfig, W1, att_src1, att_dst1, bias1,
                      W2, att_src2, att_dst2, bias2, lin_w, lin_b):
    """Fold attention vectors; apply the feature interleave permutation.

    Returns the big-weight blob [768, 512] (w1 rows 0:256, w2 rows 256:768)
    to be sharded over cores, plus small replicated tensors.
    """
    d, h, hid = cfg.d, cfg.heads, cfg.hid
    pi = _feat_perm(cfg)

    def ablock(att_s, att_d):
        A = np.zeros((d, 2 * h), dtype=np.float64)
        for hh in range(h):
            A[hh * hid:(hh + 1) * hid, hh] = att_s[hh]
            A[hh * hid:(hh + 1) * hid, h + hh] = att_d[hh]
        return A

    A1 = ablock(att_src1.astype(np.float64), att_dst1.astype(np.float64))
    A2 = ablock(att_src2.astype(np.float64), att_dst2.astype(np.float64))
    W1A = (W1.astype(np.float64) @ A1).astype(np.float32)       # x-space
    W2A = (W2.astype(np.float64) @ A2).astype(np.float32)[pi, :]
    b1A = (bias1.astype(np.float64) @ A1).astype(np.float32).reshape(1, 2 * h)
    b2A = (bias2.astype(np.float64) @ A2).astype(np.float32).reshape(1, 2 * h)

    wblob = np.concatenate([W1[:, pi].astype(BF16),
                            W2[pi][:, pi].astype(BF16)], axis=0)   # [768, 512]
    return dict(
        wblob=wblob,
        w1a=W1A.astype(BF16), w2a=W2A.astype(BF16),
        b1=bias1[pi].reshape(1, d).astype(BF16), b1a=b1A.astype(BF16),
        b2=bias2[pi].reshape(1, d).astype(BF16), b2a=b2A.astype(BF16),
        lin_w=lin_w[pi, :].astype(BF16),
        lin_b=lin_b.reshape(1, cfg.classes).astype(np.float32),
    )


# --------------------------------------------------------------------------
# Device kernel
# --------------------------------------------------------------------------

@with_exitstack
def gat_tile_kernel(ctx: ExitStack, tc: tile.TileContext, cfg: GATConfig,
                    tpb: int, outs, ins):
    nc = tc.nc
    d, h2, nb, nloc, tblw = cfg.d, 2 * cfg.heads, cfg.nb, cfg.nloc, cfg.tblw
    ct, dt_ = cfg.ct, cfg.dt_
    epb = tpb * P
    slot = epb // 16                    # idx cols per block
    ntbl = cfg.ncore * nloc
    H = cfg.heads
    ngrp = (tpb + GRP - 1) // GRP
    wshard = WBR // cfg.ncore           # 96 weight-blob rows per core

    (o_out,) = outs
    i = ins
    ablate = os.environ.get("GAT_ABLATE", "")

    nc.gpsimd.load_library(library_config.mlp)

    # ---------------- persistent pools ----------------
    pc = ctx.enter_context(tc.tile_pool(name="consts", bufs=1))
    dram = ctx.enter_context(tc.tile_pool(name="dram", bufs=1, space="DRAM"))
    groups = [list(range(cfg.ncore))]

    lay, _ = blob_layout(cfg, tpb)
    blob = i["blob"]

    def bview(name, p, w):
        """AP view [p, w] over the blob rows of `name` (flat contiguous)."""
        r0, rows = lay[name]
        if p * w == rows * 512:
            return blob[r0:r0 + rows, :].rearrange("a b -> (a b)")                                         .rearrange("(p w) -> p w", p=p)
        assert rows == 1 and p * w <= 512
        if p == 1:
            return blob[r0:r0 + 1, 0:w]
        return blob[r0:r0 + 1, 0:p * w].rearrange("a b -> (a b)")                                        .rearrange("(p w) -> p w", p=p)

    # weight blob: shard -> AllGather -> load tiles from DRAM
    wloc = dram.tile([wshard, d], DT.bfloat16, tag="wloc")
    nc.sync.dma_start(wloc[:], bview("wsh", wshard, d))
    wfull = dram.tile([WBR, d], DT.bfloat16, tag="wfull", addr_space="Shared")
    if cfg.ncore == 1 or "nocc" in ablate:
        nc.sync.dma_start(wfull[:wshard, :], wloc[:])
    else:
        nc.gpsimd.collective_compute(
            "AllGather", ALU.bypass, replica_groups=groups,
            ins=[wloc[:].opt()], outs=[wfull[:].opt()])

    def load_const(ap_in, shape, dtype, name):
        t = pc.tile(shape, dtype, tag=name, name=name)
        nc.sync.dma_start(t[:], ap_in)
        return t

    xr0, _ = lay["x_t"]
    xrt = nloc // 8                       # blob rows per x tile (fp8)
    xt_c = [load_const(blob[xr0 + k * xrt:xr0 + (k + 1) * xrt, :]
                       .rearrange("a b -> (a b)").rearrange("(p w) -> p w", p=P),
                       [P, nloc // 2], DT.bfloat16, f"xt{k}") for k in range(ct)]
    xt = [t[:].bitcast(DT.float8e4) for t in xt_c]
    w1 = [load_const(wfull[k * P:(k + 1) * P, :], [P, d], DT.bfloat16,
                     f"w1_{k}") for k in range(ct)]
    w2 = [load_const(wfull[cfg.in_dim + k * P:cfg.in_dim + (k + 1) * P, :],
                     [P, d], DT.bfloat16, f"w2_{k}") for k in range(dt_)]
    w1a_r0, _ = lay["w1a"]
    w1a = [load_const(blob[w1a_r0 + k * 4:w1a_r0 + (k + 1) * 4, :]
                      .rearrange("a b -> (a b)").rearrange("(p w) -> p w", p=P),
                      [P, h2], DT.bfloat16, f"w1a_{k}") for k in range(ct)]
    w2a_r0, _ = lay["w2a"]
    w2a = [load_const(blob[w2a_r0 + k * 4:w2a_r0 + (k + 1) * 4, :]
                      .rearrange("a b -> (a b)").rearrange("(p w) -> p w", p=P),
                      [P, h2], DT.bfloat16, f"w2a_{k}") for k in range(dt_)]
    b1 = load_const(bview("b1", 1, d), [1, d], DT.bfloat16, "b1")
    b1a = load_const(bview("b1a", 1, h2), [1, h2], DT.bfloat16, "b1a")
    b2 = load_const(bview("b2", 1, d), [1, d], DT.bfloat16, "b2")
    b2a = load_const(bview("b2a", 1, h2), [1, h2], DT.bfloat16, "b2a")
    lw_r0, _ = lay["lin_w"]
    lin_w16 = [load_const(blob[lw_r0 + k * 4:lw_r0 + (k + 1) * 4, :]
                          .rearrange("a b -> (a b)")
                          .rearrange("(p w) -> p w", p=P),
                          [P, 16], DT.bfloat16, f"lw{k}") for k in range(dt_)]
    lin_w = [t[:, 0:cfg.classes] for t in lin_w16]
    lin_bt = load_const(bview("lin_b", 1, 2 * cfg.classes), [1, 2 * cfg.classes],
                        DT.bfloat16, "lin_bt")
    lin_b = lin_bt[:].bitcast(DT.float32)
    iota_row = load_const(bview("iota_row", P, P), [P, P], DT.bfloat16,
                          "iota_row")
    iota_col = load_const(bview("iota_col", P, 1), [P, 1], DT.bfloat16,
                          "iota_col")
    drc = load_const(bview("drc", P, nb * tpb), [P, nb * tpb], DT.bfloat16,
                     "drc")
    bid_t = load_const(bview("bid", P, nb * 2), [P, nb * 2], DT.bfloat16,
                       "bid_t")
    wcol_t = load_const(bview("wcol", P, nb * 2), [P, nb * 2], DT.bfloat16,
                        "wcol_t")
    drow = bview("drow", 1, nb * epb)

    # replicate the 16 unique gather-idx rows to 128 partitions (bf16
    # container of int16 bits; bitcast at use)
    g_idx = pc.tile([P, nb * slot], DT.bfloat16, tag="g_idx")
    for k in range(8):
        nc.sync.dma_start(g_idx[k * 16:(k + 1) * 16, :],
                          bview("g16", 16, nb * slot))

    ones_bf = pc.tile([1, P], DT.bfloat16, tag="ones_bf")
    nc.vector.memset(ones_bf[:], 1.0)
    iota_colf = pc.tile([P, 1], DT.float32, tag="iota_colf")
    nc.vector.tensor_copy(iota_colf[:], iota_col[:])
    id_bf = pc.tile([P, P], DT.bfloat16, tag="id_bf")
    nc.vector.tensor_tensor(out=id_bf[:], in0=iota_row[:],
                            in1=iota_col[:].to_broadcast([P, P]), op=ALU.is_equal)
    id_f32 = pc.tile([P, P], DT.float32, tag="id_f32")
    nc.vector.tensor_copy(id_f32[:], id_bf[:])
    zrow_bf = pc.tile([1, tblw], DT.bfloat16, tag="zrow")
    nc.vector.memset(zrow_bf[:], 0.0)
    ones64_bf = pc.tile([1, cfg.g], DT.bfloat16, tag="ones64_bf")
    nc.vector.memset(ones64_bf[:], 1.0)
    lin_b_bf = pc.tile([1, cfg.classes], DT.bfloat16, tag="lin_b_bf")
    nc.vector.tensor_copy(lin_b_bf[:], lin_b)

    # a_dst per layer, kept in SBUF (bf16): [128, nb*h]
    adst_bf = pc.tile([P, nb * H], DT.bfloat16, tag="adst1")
    adst2_bf = pc.tile([P, nb * H], DT.bfloat16, tag="adst2")
    # h^T (bf16) for layer-2 matmuls: [128, dt_*nloc]
    hT = pc.tile([P, dt_ * nloc], DT.bfloat16, tag="hT")

    # DRAM tables
    loc_tbl1 = dram.tile([nloc, tblw], DT.bfloat16, tag="ltbl1")
    full_tbl1 = dram.tile([ntbl, tblw], DT.bfloat16, tag="ftbl1",
                          addr_space="Shared")
    loc_tbl2 = dram.tile([nloc, tblw], DT.bfloat16, tag="ltbl2")
    full_tbl2 = dram.tile([ntbl, tblw], DT.bfloat16, tag="ftbl2",
                          addr_space="Shared")

    # persistent pooling PSUM accumulator [128, dt_*G]
    ppool = ctx.enter_context(tc.tile_pool(name="ppool", bufs=1, space="PSUM"))
    p_pool = ppool.tile([P, dt_ * cfg.g], DT.float32, tag="p_pool")

    # ---------------- node phases ----------------
    def node_phase(layer):
        with tc.tile_pool(name=f"np{layer}", bufs=3) as sb, \
             tc.tile_pool(name=f"npp{layer}", bufs=2, space="PSUM") as ps:
            loc_tbl = loc_tbl1 if layer == 1 else loc_tbl2
            adst = adst_bf if layer == 1 else adst2_bf
            for k in range(nb):
                pxw = ps.tile([P, d], DT.float32, tag="pxw")
                pa = ps.tile([P, h2], DT.float32, tag="pa")
                if layer == 1:
                    lhs = [xt[c][:, k * P:(k + 1) * P] for c in range(ct)]

                    ws, was, bias, biasa = w1, w1a, b1, b1a
                else:
                    lhs = [hT[:, c * nloc + k * P:c * nloc + (k + 1) * P]
                           for c in range(dt_)]
                    ws, was, bias, biasa = w2, w2a, b2, b2a
                for c in range(len(lhs)):
                    nc.tensor.matmul(pxw[:], lhsT=lhs[c], rhs=ws[c][:],
                                     start=(c == 0), stop=False)
                    nc.tensor.matmul(pa[:], lhsT=lhs[c], rhs=was[c][:],
                                     start=(c == 0), stop=False)
                nc.tensor.matmul(pxw[:], lhsT=ones_bf[:], rhs=bias[:],
                                 start=False, stop=True)
                nc.tensor.matmul(pa[:], lhsT=ones_bf[:], rhs=biasa[:],
                                 start=False, stop=True)
                tbl = sb.tile([P, tblw], DT.bfloat16, tag="tbl")
                nc.scalar.copy(tbl[:, 0:d], pxw[:])
                nc.scalar.copy(tbl[:, d:d + H], pa[:, 0:H])
                nc.scalar.copy(tbl[:, d + H:tblw],
                               pa[:, 0:1].to_broadcast([P, tblw - d - H]))
                nc.vector.tensor_copy(adst[:, k * H:(k + 1) * H], pa[:, H:h2])
                nc.sync.dma_start(loc_tbl[k * P:(k + 1) * P, :], tbl[:])
            nc.sync.dma_start(loc_tbl[nloc - 1:nloc, :], zrow_bf[:])

    # ---------------- edge phases ----------------
    def edge_phase(layer):
        full_tbl = full_tbl1 if layer == 1 else full_tbl2
        adst = adst_bf if layer == 1 else adst2_bf
        ptr_bufs = 1 if layer == 1 else 2
        with tc.tile_pool(name=f"ep_g{layer}", bufs=2) as gp, \
             tc.tile_pool(name=f"ep_m{layer}", bufs=2) as mp, \
             tc.tile_pool(name=f"ep_s{layer}", bufs=2) as sb, \
             tc.tile_pool(name=f"ep_t{layer}", bufs=2) as tp, \
             tc.tile_pool(name=f"ep_pb{layer}", bufs=2, space="PSUM") as psb, \
             tc.tile_pool(name=f"ep_pa{layer}", bufs=2, space="PSUM") as psa, \
             tc.tile_pool(name=f"ep_po{layer}", bufs=2, space="PSUM") as pso:
            for b in range(nb):
                gath = gp.tile([P, tpb, tblw], DT.bfloat16, tag="gath")
                nc.gpsimd.dma_gather(
                    gath[:], full_tbl[:],
                    g_idx[:, b * slot:(b + 1) * slot].bitcast(DT.int16),
                    epb, epb, tblw, single_packet=False)
                bcr = gp.tile([1, epb], DT.bfloat16, tag="bcr")
                nc.sync.dma_start(bcr[:], drow[:, b * epb:(b + 1) * epb])

                # one-hot builds:
                #   m  [e, dst] from drc (batched is_equal)
                #   mt [dst, e] via PE row-broadcast of drow + is_equal
                m_t = mp.tile([P, epb], DT.bfloat16, tag="m_t")
                nc.vector.tensor_tensor(
                    out=m_t[:].rearrange("p (t c) -> p t c", t=tpb),
                    in0=iota_row[:].unsqueeze(1).to_broadcast([P, tpb, P]),
                    in1=drc[:, b * tpb:(b + 1) * tpb]
                        .unsqueeze(2).to_broadcast([P, tpb, P]),
                    op=ALU.is_equal)
                mt_t = mp.tile([P, epb], DT.bfloat16, tag="mt_t")
                pa_ps = psa.tile([P, tpb * H + H], DT.float32, tag="pa_ps")
                p_s = pa_ps[:, tpb * H:tpb * H + H]
                for g in range(ngrp):
                    t0 = g * GRP
                    gw = min(GRP, tpb - t0)
                    nw = gw * P
                    pbc = psb.tile([P, GRP * P], DT.float32, tag="pbc")
                    nc.tensor.matmul(pbc[:, 0:nw], lhsT=ones_bf[:],
                                     rhs=bcr[:, t0 * P:t0 * P + nw],
                                     start=True, stop=True)
                    pbcs = sb.tile([P, GRP * P], DT.bfloat16, tag="pbcs")
                    nc.scalar.copy(pbcs[:, 0:nw], pbc[:, 0:nw])
                    nc.vector.tensor_scalar(
                        out=mt_t[:, t0 * P:t0 * P + nw], in0=pbcs[:, 0:nw],
                        scalar1=iota_colf[:], scalar2=None, op0=ALU.is_equal)
                    for t in range(gw):
                        nc.tensor.matmul(
                            pa_ps[:, (t0 + t) * H:(t0 + t + 1) * H],
                            lhsT=mt_t[:, (t0 + t) * P:(t0 + t + 1) * P],
                            rhs=adst[:, b * H:(b + 1) * H],
                            start=True, stop=True)

                # e = lrelu(asrc + adst) ; ex = exp(e) (bf16)
                asrc_f = sb.tile([P, tpb * H], DT.float32, tag="asrc")
                nc.scalar.copy(
                    asrc_f[:].rearrange("p (a b) -> p a b", a=tpb),
                    gath[:, :, d:d + H])
                e_sum = sb.tile([P, tpb * H], DT.float32, tag="esum")
                nc.vector.tensor_tensor(out=e_sum[:], in0=asrc_f[:],
                                        in1=pa_ps[:, 0:tpb * H], op=ALU.add)
                e_lr = sb.tile([P, tpb * H], DT.float32, tag="elr")
                nc.vector.scalar_tensor_tensor(
                    out=e_lr[:], in0=e_sum[:], scalar=cfg.neg_slope,
                    in1=e_sum[:], op0=ALU.mult, op1=ALU.max)
                ex_b = sb.tile([P, tpb * H], DT.bfloat16, tag="exb")
                nc.scalar.activation(ex_b[:], e_lr[:], AF.Exp)

                # pass B: msg = feat * ex (DVE 2x, head-fast), segment sums
                p_out = pso.tile([P, d], DT.float32, tag="p_out")
                for g in range(ngrp):
                    t0 = g * GRP
                    gw = min(GRP, tpb - t0)
                    msg = mp.tile([P, GRP, d], DT.bfloat16, tag="msg")
                    nc.vector.tensor_tensor(
                        out=msg[:, 0:gw, :].rearrange("p t (c h) -> p t c h", h=H),
                        in0=gath[:, t0:t0 + gw, 0:d]
                            .rearrange("p t (c h) -> p t c h", h=H),
                        in1=ex_b[:, t0 * H:(t0 + gw) * H]
                            .rearrange("p (t h) -> p t h", t=gw)
                            .unsqueeze(2).to_broadcast([P, gw, cfg.hid, H]),
                        op=ALU.mult)
                    for t in range(gw):
                        tt = t0 + t
                        nc.tensor.matmul(p_s, lhsT=m_t[:, tt * P:(tt + 1) * P],
                                         rhs=ex_b[:, tt * H:(tt + 1) * H],
                                         start=(tt == 0), stop=(tt == tpb - 1))
                        nc.tensor.matmul(p_out[:], lhsT=m_t[:, tt * P:(tt + 1) * P],
                                         rhs=msg[:, t, :],
                                         start=(tt == 0), stop=(tt == tpb - 1))

                # normalize + elu
                s_g = sb.tile([P, H], DT.float32, tag="sg")
                nc.vector.tensor_scalar_max(s_g[:], p_s, 1e-30)
                rs = sb.tile([P, H], DT.float32, tag="rs")
                nc.vector.reciprocal(rs[:], s_g[:])
                outn = tp.tile([P, d], DT.float32, tag="outn")
                nc.vector.tensor_tensor(
                    out=outn[:].rearrange("p (a b) -> p a b", a=cfg.hid),
                    in0=p_out[:].rearrange("p (a b) -> p a b", a=cfg.hid),
                    in1=rs[:].unsqueeze(1).to_broadcast([P, cfg.hid, H]),
                    op=ALU.mult)
                mn = tp.tile([P, d], DT.float32, tag="mn")
                nc.vector.tensor_scalar_min(mn[:], outn[:], 0.0)
                ee = tp.tile([P, d], DT.float32, tag="ee")
                nc.scalar.activation(ee[:], mn[:], AF.Exp)
                h_f = tp.tile([P, d], DT.float32, tag="hf")
                nc.vector.scalar_tensor_tensor(
                    out=h_f[:], in0=ee[:], scalar=-1.0, in1=outn[:],
                    op0=ALU.add, op1=ALU.max)
                h_b = tp.tile([P, d], DT.bfloat16, tag="hb")
                nc.vector.tensor_copy(h_b[:], h_f[:])

                if layer == 1:
                    for c in range(dt_):
                        ptr = psb.tile([P, P], DT.bfloat16, tag="ptr",
                                       bufs=ptr_bufs)
                        nc.tensor.transpose(ptr[:], h_b[:, c * P:(c + 1) * P],
                                            id_bf[:])
                        nc.scalar.copy(
                            hT[:, c * nloc + b * P:c * nloc + (b + 1) * P],
                            ptr[:])
                else:
                    mbb = tp.tile([P, cfg.g], DT.bfloat16, tag="mbb")
                    nc.vector.tensor_scalar(
                        out=mbb[:], in0=iota_row[:, 0:cfg.g],
                        scalar1=bid_t[:, 2 * b:2 * b + 2].bitcast(DT.float32),
                        scalar2=wcol_t[:, 2 * b:2 * b + 2].bitcast(DT.float32),
                        op0=ALU.is_equal, op1=ALU.mult)
                    for c in range(dt_):
                        nc.tensor.matmul(
                            p_pool[:, c * cfg.g:(c + 1) * cfg.g],
                            lhsT=h_b[:, c * P:(c + 1) * P],
                            rhs=mbb[:],
                            start=(b == 0 and c == 0),
                            stop=(b == nb - 1 and c == dt_ - 1))

    def gather_table(loc, full):
        if cfg.ncore == 1 or "nocc" in ablate:
            nc.sync.dma_start(full[:cfg.nloc, :], loc[:])
        else:
            nc.gpsimd.collective_compute(
                "AllGather", ALU.bypass, replica_groups=groups,
                ins=[loc[:].opt()], outs=[full[:].opt()])

    # ---------------- run phases ----------------
    if "nonp" not in ablate:
        node_phase(1)
    gather_table(loc_tbl1, full_tbl1)
    if "noedge" not in ablate:
        edge_phase(1)
    else:
        with tc.tile_pool(name="abl", bufs=1) as ab:
            nc.vector.memset(hT[:, 0:P], 0.0)
            pzf = ab.tile([P, P], DT.float32, tag="pzf")
            nc.vector.memset(pzf[:], 0.0)
            zr64 = ab.tile([P, cfg.g], DT.float32, tag="zr64")
            nc.vector.memset(zr64[:], 0.0)
            for c in range(dt_):
                nc.tensor.matmul(p_pool[:, c * cfg.g:(c + 1) * cfg.g],
                                 lhsT=pzf[:], rhs=zr64[:],
                                 start=(c == 0), stop=(c == dt_ - 1))
    if "nonp" not in ablate:
        node_phase(2)
    gather_table(loc_tbl2, full_tbl2)
    if "noedge" not in ablate:
        edge_phase(2)

    # ---------------- pooling reduce + classifier ----------------
    with tc.tile_pool(name="fin", bufs=1) as sb, \
         tc.tile_pool(name="finp", bufs=1, space="PSUM") as ps:
        pool_sb = sb.tile([P, dt_ * cfg.g], DT.float32, tag="pool_sb")
        nc.vector.tensor_copy(pool_sb[:], p_pool[:])
        pool_g0 = sb.tile([P, dt_ * cfg.g], DT.float32, tag="pool_g0")
        if cfg.ncore == 1 or "nocc" in ablate:
            nc.vector.tensor_copy(pool_g0[:], pool_sb[:])
        else:
            pool_l = dram.tile([P, dt_ * cfg.g], DT.float32, tag="pool_l")
            pool_r = dram.tile([P, dt_ * cfg.g], DT.float32, tag="pool_r")
            nc.sync.dma_start(pool_l[:], pool_sb[:])
            nc.gpsimd.collective_compute(
                "AllReduce", ALU.add, replica_groups=groups,
                ins=[pool_l[:].opt()], outs=[pool_r[:].opt()])
            nc.sync.dma_start(pool_g0[:], pool_r[:])
        pool_gb = sb.tile([P, dt_ * cfg.g], DT.bfloat16, tag="pool_gb")
        nc.vector.tensor_copy(pool_gb[:], pool_g0[:])

        p_lg = ps.tile([cfg.classes, cfg.g], DT.float32, tag="p_lg")
        for c in range(dt_):
            nc.tensor.matmul(p_lg[:], lhsT=lin_w[c],
                             rhs=pool_gb[:, c * cfg.g:(c + 1) * cfg.g],
                             start=(c == 0), stop=False)
        nc.tensor.matmul(p_lg[:], lhsT=lin_b_bf[:], rhs=ones64_bf[:],
                         start=False, stop=True)
        lg_sb = sb.tile([cfg.classes, cfg.g], DT.float32, tag="lg_sb")
        nc.vector.tensor_copy(lg_sb[:], p_lg[:])
        p_t = ps.tile([cfg.g, cfg.classes], DT.float32, tag="p_t")
        nc.tensor.transpose(p_t[:], lg_sb[:], id_f32[:cfg.classes, :cfg.classes])
        logit = sb.tile([cfg.g, cfg.classes], DT.float32, tag="logit")
        nc.vector.tensor_copy(logit[:], p_t[:])

        rmax = sb.tile([cfg.g, 1], DT.float32, tag="rmax")
        nc.vector.reduce_max(rmax[:], logit[:], axis=mybir.AxisListType.X)
        sh = sb.tile([cfg.g, cfg.classes], DT.float32, tag="sh")
        nc.vector.tensor_scalar(out=sh[:], in0=logit[:], scalar1=rmax[:],
                                scalar2=None, op0=ALU.subtract)
        exps = sb.tile([cfg.g, cfg.classes], DT.float32, tag="exps")
        nc.scalar.activation(exps[:], sh[:], AF.Exp)
        ssum = sb.tile([cfg.g, 1], DT.float32, tag="ssum")
        nc.vector.reduce_sum(ssum[:], exps[:], axis=mybir.AxisListType.X)
        lns = sb.tile([cfg.g, 1], DT.float32, tag="lns")
        nc.scalar.activation(lns[:], ssum[:], AF.Ln)
        res = sb.tile([cfg.g, cfg.classes], DT.float32, tag="res")
        nc.vector.tensor_scalar(out=res[:], in0=sh[:], scalar1=lns[:],
                                scalar2=None, op0=ALU.subtract)
        nc.sync.dma_start(o_out[:], res[:])


# --------------------------------------------------------------------------
# Program build + run
# --------------------------------------------------------------------------

def build_program(cfg: GATConfig, tpb: int):
    from concourse import bacc
    nc = bacc.Bacc("TRN2", target_bir_lowering=False, debug=False,
                   num_devices=cfg.ncore)
    nb, nloc, h2 = cfg.nb, cfg.nloc, 2 * cfg.heads
    epb = tpb * P
    ins = {}

    def inp(name, shape, dt):
        ins[name] = nc.dram_tensor(name, list(shape), dt, kind="ExternalInput").ap()

    _, R = blob_layout(cfg, tpb)
    inp("blob", [R, 512], DT.bfloat16)

    out_ap = nc.dram_tensor("out", [cfg.g, cfg.classes], DT.float32,
                            kind="ExternalOutput").ap()

    with tile.TileContext(nc) as tc:
        gat_tile_kernel(tc, cfg, tpb, [out_ap], ins)
    nc.compile()
    return nc


_CACHE = {}


def _prepare(cfg: GATConfig, inputs):
    import hashlib
    edge_index = np.asarray(inputs["edge_index"])
    batch = np.asarray(inputs["batch"])
    key = hashlib.sha1(edge_index.tobytes() + batch.tobytes()).hexdigest()
    if key in _CACHE:
        return _CACHE[key]
    tpb, cores, consts = build_host_data(cfg, edge_index, batch)
    nc = build_program(cfg, tpb)
    _CACHE[key] = (nc, tpb, cores, consts)
    return _CACHE[key]


def make_in_maps(cfg: GATConfig, inputs, cores, consts):
    wd = build_weight_data(cfg, inputs["W1"], inputs["att_src1"], inputs["att_dst1"],
                           inputs["bias1"], inputs["W2"], inputs["att_src2"],
                           inputs["att_dst2"], inputs["bias2"], inputs["lin_w"],
                           inputs["lin_b"])
    F8 = ml_dtypes.float8_e4m3
    x = np.asarray(inputs["x"], dtype=np.float32)
    x_t_full = np.ascontiguousarray(x.T).astype(F8)     # [in_dim, n] fp8
    node_at = consts["node_at"]
    wshard = WBR // cfg.ncore
    tpb = consts["tpb"]
    lay, R = blob_layout(cfg, tpb)

    def put(blob, name, arr_bf16_flat):
        r0, rows = lay[name]
        flat = np.ascontiguousarray(arr_bf16_flat).reshape(-1)
        assert flat.size <= rows * 512, (name, flat.size, rows)
        blob.reshape(-1)[r0 * 512:r0 * 512 + flat.size] = flat

    lwp = np.zeros((cfg.d, 16), dtype=BF16)
    lwp[:, :cfg.classes] = wd["lin_w"]
    iota_row = np.tile(np.arange(P, dtype=np.float32).reshape(1, P),
                       (P, 1)).astype(BF16)
    iota_col = np.arange(P, dtype=np.float32).astype(BF16)

    in_maps = []
    for c in range(cfg.ncore):
        xt = np.zeros((cfg.in_dim, cfg.nloc), dtype=F8)
        nodes = node_at[c]
        real = nodes >= 0
        xt[:, real] = x_t_full[:, nodes[real]]
        blob = np.zeros((R, 512), dtype=BF16)
        put(blob, "x_t", xt.view(np.uint8).reshape(-1, 2).view(np.uint16).view(BF16))
        put(blob, "wsh", wd["wblob"][c * wshard:(c + 1) * wshard])
        put(blob, "w1a", wd["w1a"])
        put(blob, "w2a", wd["w2a"])
        put(blob, "b1", wd["b1"])
        put(blob, "b2", wd["b2"])
        put(blob, "b1a", wd["b1a"])
        put(blob, "b2a", wd["b2a"])
        put(blob, "lin_w", lwp)
        put(blob, "iota_row", iota_row)
        put(blob, "iota_col", iota_col)
        put(blob, "drc", cores[c]["drc"])
        put(blob, "drow", cores[c]["drow"])
        put(blob, "g16", cores[c]["g16"].view(BF16))
        put(blob, "bid", cores[c]["bid"].view(BF16))
        put(blob, "wcol", cores[c]["wcol"].view(BF16))
        put(blob, "lin_b", wd["lin_b"].view(BF16))
        in_maps.append(dict(blob=blob))
    return in_maps


def run(cfg: GATConfig, inputs, trace=False):
    from concourse.bass_utils import run_bass_kernel_spmd
    nc, tpb, cores, consts = _prepare(cfg, inputs)
    in_maps = make_in_maps(cfg, inputs, cores, consts)
    res = run_bass_kernel_spmd(nc, in_maps, core_ids=list(range(cfg.ncore)),
                               trace=trace)
    return res


def kernel(**inputs) -> np.ndarray:
    res = run(CFG, inputs, trace=False)
    return np.asarray(res.results[0]["out"])


# revision 21
# speedup vs baseline: 10.4832x; 10.4832x over previous
"""GAT (2-layer, 8-head) Trainium2 Bass kernel, 8-way node-sharded. v4.

Per-execution input staging costs ~0.7 ms/MB/core on this stack, so inputs
are aggressively minimized (~1.7 MB/core):
  - x is shipped bf16, transposed, per-core shard only.
  - The two big weight matrices (W1|W2 -> [768,512] bf16) are SHARDED over
    cores and AllGathered on-device (98 KB/core instead of 786 KB).
  - The gather index table is shipped as its unique 16 rows and replicated
    to 128 partitions on-device.
  - Pooling one-hot is built on-device from per-node batch-id + 1/cnt
    scalars (10 KB instead of 655 KB).
  - One-hot edge matrices are built ON-CHIP from compact dst_rel arrays
    (drc/drow, 87 KB each).

Compute layout:
  - Host balances nodes across the 160 blocks by in-degree (free node
    permutation) -> tpb drops 19->17 and all cores have identical loads.
  - Feature axis interleaved (c-major, head-fast) so the per-edge attention
    multiply (msg) runs in DVE 2x mode.
  - Per layer: node matmuls (bf16) -> table [feat 512 | a_src 8 | pad],
    AllGather, edge phase: dma_gather per-edge rows; one-hot matmuls on the
    TensorEngine for segment softmax+aggregation (denominators cancel per
    row, normalize once per block).  LeakyReLU / ELU fused via
    scalar_tensor_tensor.  Layer-1 h transposed on-chip via PE transposes
    into SBUF for layer-2 node matmuls.  Mean-pool accumulates in a
    persistent PSUM bank, AllReduce, classifier + log_softmax on-chip.
"""
import os
import sys
import heapq
from contextlib import ExitStack
from dataclasses import dataclass

import numpy as np

sys.path.insert(0, "/opt/trn_rl_repo")

import ml_dtypes  # noqa: E402

import concourse.bass as bass  # noqa: E402
import concourse.tile as tile  # noqa: E402
from concourse import mybir  # noqa: E402
from concourse import library_config  # noqa: E402
from concourse._compat import with_exitstack  # noqa: E402

P = 128
AF = mybir.ActivationFunctionType
ALU = mybir.AluOpType
DT = mybir.dt
BF16 = ml_dtypes.bfloat16

GRP = 4          # pass A/B tile batching (tiles per group)
WBR = 768        # weight blob rows (w1 256 + w2 512)


@dataclass(frozen=True)
class GATConfig:
    n: int = 20000
    e: int = 320000
    in_dim: int = 256
    hid: int = 64
    heads: int = 8
    classes: int = 10
    g: int = 64
    ncore: int = 8
    neg_slope: float = 0.2

    @property
    def d(self):
        return self.hid * self.heads          # 512

    @property
    def nb(self):
        return 20                             # node blocks / core

    @property
    def nloc(self):
        return self.nb * P                    # 2560 padded local rows

    @property
    def tblw(self):
        return self.d + P                     # 640 bf16 -> 1280B rows

    @property
    def ct(self):
        return self.in_dim // P               # contraction tiles layer 1

    @property
    def dt_(self):
        return self.d // P                    # d tiles (4)


CFG = GATConfig()


def _feat_perm(cfg: GATConfig):
    """new feature j=(c,h) <- old feature h*hid+c (head-fast interleave)."""
    j = np.arange(cfg.d)
    return (j % cfg.heads) * cfg.hid + j // cfg.heads


def blob_layout(cfg: GATConfig, tpb: int):
    """Row layout of the single packed bf16 input tensor [R, 512].

    Each entry: name -> (row0, nrows).  All multi-partition views are
    contiguous flat regions reshaped on-device via AP rearrange.
    """
    epb = tpb * P
    nb = cfg.nb
    sizes = [
        ("x_t", cfg.in_dim * cfg.nloc),        # [256,2560] -> 2 tiles [128,2560]
        ("wsh", (WBR // cfg.ncore) * cfg.d),   # weight-blob shard rows
        ("w1a", cfg.in_dim * 16),
        ("w2a", cfg.d * 16),
        ("b1", cfg.d),
        ("b2", cfg.d),
        ("b1a", 512),                          # 16 used
        ("b2a", 512),
        ("lin_w", cfg.d * 16),                 # width padded 10->16
        ("iota_row", P * P),
        ("iota_col", 512),                     # 128 used
        ("drc", P * nb * tpb),
        ("drow", nb * epb),
        ("g16", 16 * (nb * epb // 16)),        # int16 bits in bf16 container
        ("bid", P * nb * 2),                   # fp32 bits
        ("wcol", P * nb * 2),                  # fp32 bits
        ("lin_b", 512),                        # fp32 bits, 20 bf16 els used
    ]
    out = {}
    r = 0
    for name, els in sizes:
        rows = (els + 511) // 512
        out[name] = (r, rows)
        r += rows
    return out, r


# --------------------------------------------------------------------------
# Host-side preprocessing
# --------------------------------------------------------------------------

def build_host_data(cfg: GATConfig, edge_index: np.ndarray, batch: np.ndarray):
    """Balance nodes over blocks, build per-core compact index arrays."""
    n, ncore, nb, nloc = cfg.n, cfg.ncore, cfg.nb, cfg.nloc
    nblocks = ncore * nb
    src = np.concatenate([edge_index[0].astype(np.int64), np.arange(n, dtype=np.int64)])
    dst = np.concatenate([edge_index[1].astype(np.int64), np.arange(n, dtype=np.int64)])
    deg = np.bincount(dst, minlength=n)

    # greedy degree balancing: heaviest node -> lightest non-full block
    cap = np.full(nblocks, P, dtype=np.int64)
    for c in range(ncore):
        cap[c * nb + nb - 1] = P - 1          # reserve per-core zero row
    order = np.argsort(-deg, kind="stable")
    counts = np.zeros(nblocks, dtype=np.int64)
    totals = np.zeros(nblocks, dtype=np.int64)
    blk_of = np.empty(n, dtype=np.int64)
    slot_in_blk = np.empty(n, dtype=np.int64)
    heap = [(0, b) for b in range(nblocks)]
    heapq.heapify(heap)
    for v in order:
        while True:
            tot, b = heapq.heappop(heap)
            if counts[b] < cap[b]:
                break
        blk_of[v] = b
        slot_in_blk[v] = counts[b]
        counts[b] += 1
        totals[b] += deg[v]
        if counts[b] < cap[b]:
            heapq.heappush(heap, (int(totals[b]), b))

    tpb = int(np.ceil(totals.max() / P))
    epb = tpb * P

    node_core = blk_of // nb
    local_blk = blk_of % nb
    local_slot = local_blk * P + slot_in_blk
    row_id = node_core * nloc + local_slot

    node_at = np.full((ncore, nloc), -1, dtype=np.int64)
    node_at[node_core, local_slot] = np.arange(n)

    cnt_g = np.bincount(batch, minlength=cfg.g).astype(np.float64)
    inv_cnt = (1.0 / np.maximum(cnt_g, 1.0)).astype(np.float32)

    e_core = node_core[dst]
    e_blk = local_blk[dst]
    e_rel = slot_in_blk[dst]

    cores = []
    for c in range(ncore):
        m = e_core == c
        es, eb, er = src[m], e_blk[m], e_rel[m]
        o = np.argsort(eb, kind="stable")
        es, eb, er = es[o], eb[o], er[o]
        cnts = np.bincount(eb, minlength=nb)
        zrow_gid = c * nloc + nloc - 1

        src_tid = np.full((nb, epb), zrow_gid, dtype=np.int64)
        dst_rel = np.full((nb, epb), 255, dtype=np.int64)
        off = 0
        for b in range(nb):
            k = int(cnts[b])
            sl = slice(off, off + k)
            src_tid[b, :k] = row_id[es[sl]]
            dst_rel[b, :k] = er[sl]
            off += k
        assert src_tid.max() < 2 ** 15

        # dma_gather idx, unique 16 rows: [16, nb*epb/16] int16
        g16 = np.zeros((16, nb * epb // 16), dtype=np.int16)
        for b in range(nb):
            g16[:, b * (epb // 16):(b + 1) * (epb // 16)] = \
                src_tid[b].reshape(-1, 16).T.astype(np.int16)

        # dst_rel column form [128, nb*tpb]: A[p, b*tpb+t] = rel[b, t*128+p]
        drc = dst_rel.reshape(nb, tpb, P).transpose(2, 0, 1).reshape(P, nb * tpb)
        drc = np.ascontiguousarray(drc).astype(BF16)
        # dst_rel row form [1, nb*epb] bf16 (broadcast on device via PE)
        drow = dst_rel.reshape(1, nb * epb).astype(BF16)

        # per-slot batch id + prescaled 1/cnt: [128, nb] each
        bid = np.full((P, nb), 255.0, dtype=np.float32)
        wcol = np.zeros((P, nb), dtype=np.float32)
        for b in range(nb):
            nodes = node_at[c, b * P:(b + 1) * P]
            real = nodes >= 0
            bid[real, b] = batch[nodes[real]]
            wcol[real, b] = inv_cnt[batch[nodes[real]]]

        cores.append(dict(g16=g16, drc=drc, drow=drow,
                          bid=bid, wcol=wcol))

    consts = dict(node_at=node_at, tpb=tpb)
    return tpb, cores, consts


def build_weight_data(cfg: GATConfig, W1, att_src1, att_dst1, bias1,
                      W2, att_src2, att_dst2, bias2, lin_w, lin_b):
    """Fold attention vectors; apply the feature interleave permutation.

    Returns the big-weight blob [768, 512] (w1 rows 0:256, w2 rows 256:768)
    to be sharded over cores, plus small replicated tensors.
    """
    d, h, hid = cfg.d, cfg.heads, cfg.hid
    pi = _feat_perm(cfg)

    def ablock(att_s, att_d):
        A = np.zeros((d, 2 * h), dtype=np.float64)
        for hh in range(h):
            A[hh * hid:(hh + 1) * hid, hh] = att_s[hh]
            A[hh * hid:(hh + 1) * hid, h + hh] = att_d[hh]
        return A

    A1 = ablock(att_src1.astype(np.float64), att_dst1.astype(np.float64))
    A2 = ablock(att_src2.astype(np.float64), att_dst2.astype(np.float64))
    W1A = (W1.astype(np.float64) @ A1).astype(np.float32)       # x-space
    W2A = (W2.astype(np.float64) @ A2).astype(np.float32)[pi, :]
    b1A = (bias1.astype(np.float64) @ A1).astype(np.float32).reshape(1, 2 * h)
    b2A = (bias2.astype(np.float64) @ A2).astype(np.float32).reshape(1, 2 * h)

    wblob = np.concatenate([W1[:, pi].astype(BF16),
                            W2[pi][:, pi].astype(BF16)], axis=0)   # [768, 512]
    return dict(
        wblob=wblob,
        w1a=W1A.astype(BF16), w2a=W2A.astype(BF16),
        b1=bias1[pi].reshape(1, d).astype(BF16), b1a=b1A.astype(BF16),
        b2=bias2[pi].reshape(1, d).astype(BF16), b2a=b2A.astype(BF16),
        lin_w=lin_w[pi, :].astype(BF16),
        lin_b=lin_b.reshape(1, cfg.classes).astype(np.float32),
    )


# --------------------------------------------------------------------------
# Device kernel
# --------------------------------------------------------------------------

@with_exitstack
def gat_tile_kernel(ctx: ExitStack, tc: tile.TileContext, cfg: GATConfig,
                    tpb: int, outs, ins):
    nc = tc.nc
    d, h2, nb, nloc, tblw = cfg.d, 2 * cfg.heads, cfg.nb, cfg.nloc, cfg.tblw
    ct, dt_ = cfg.ct, cfg.dt_
    epb = tpb * P
    slot = epb // 16                    # idx cols per block
    ntbl = cfg.ncore * nloc
    H = cfg.heads
    ngrp = (tpb + GRP - 1) // GRP
    wshard = WBR // cfg.ncore           # 96 weight-blob rows per core

    (o_out,) = outs
    i = ins
    ablate = os.environ.get("GAT_ABLATE", "")

    nc.gpsimd.load_library(library_config.mlp)

    # ---------------- persistent pools ----------------
    pc = ctx.enter_context(tc.tile_pool(name="consts", bufs=1))
    dram = ctx.enter_context(tc.tile_pool(name="dram", bufs=1, space="DRAM"))
    groups = [list(range(cfg.ncore))]

    lay, _ = blob_layout(cfg, tpb)
    blob = i["blob"]

    def bview(name, p, w):
        """AP view [p, w] over the blob rows of `name` (flat contiguous)."""
        r0, rows = lay[name]
        if p * w == rows * 512:
            return blob[r0:r0 + rows, :].rearrange("a b -> (a b)")                                         .rearrange("(p w) -> p w", p=p)
        assert rows == 1 and p * w <= 512
        if p == 1:
            return blob[r0:r0 + 1, 0:w]
        return blob[r0:r0 + 1, 0:p * w].rearrange("a b -> (a b)")                                        .rearrange("(p w) -> p w", p=p)

    # weight blob: shard -> AllGather -> load tiles from DRAM
    wloc = dram.tile([wshard, d], DT.bfloat16, tag="wloc")
    nc.sync.dma_start(wloc[:], bview("wsh", wshard, d))
    wfull = dram.tile([WBR, d], DT.bfloat16, tag="wfull", addr_space="Shared")
    if cfg.ncore == 1 or "nocc" in ablate:
        nc.sync.dma_start(wfull[:wshard, :], wloc[:])
    else:
        nc.gpsimd.collective_compute(
            "AllGather", ALU.bypass, replica_groups=groups,
            ins=[wloc[:].opt()], outs=[wfull[:].opt()])

    def load_const(ap_in, shape, dtype, name):
        t = pc.tile(shape, dtype, tag=name, name=name)
        nc.sync.dma_start(t[:], ap_in)
        return t

    xr0, _ = lay["x_t"]
    xt = [load_const(blob[xr0 + k * (nloc // 4):xr0 + (k + 1) * (nloc // 4), :]
                     .rearrange("a b -> (a b)").rearrange("(p w) -> p w", p=P),
                     [P, nloc], DT.bfloat16, f"xt{k}") for k in range(ct)]
    w1 = [load_const(wfull[k * P:(k + 1) * P, :], [P, d], DT.bfloat16,
                     f"w1_{k}") for k in range(ct)]
    w2 = [load_const(wfull[cfg.in_dim + k * P:cfg.in_dim + (k + 1) * P, :],
                     [P, d], DT.bfloat16, f"w2_{k}") for k in range(dt_)]
    w1a_r0, _ = lay["w1a"]
    w1a = [load_const(blob[w1a_r0 + k * 4:w1a_r0 + (k + 1) * 4, :]
                      .rearrange("a b -> (a b)").rearrange("(p w) -> p w", p=P),
                      [P, h2], DT.bfloat16, f"w1a_{k}") for k in range(ct)]
    w2a_r0, _ = lay["w2a"]
    w2a = [load_const(blob[w2a_r0 + k * 4:w2a_r0 + (k + 1) * 4, :]
                      .rearrange("a b -> (a b)").rearrange("(p w) -> p w", p=P),
                      [P, h2], DT.bfloat16, f"w2a_{k}") for k in range(dt_)]
    b1 = load_const(bview("b1", 1, d), [1, d], DT.bfloat16, "b1")
    b1a = load_const(bview("b1a", 1, h2), [1, h2], DT.bfloat16, "b1a")
    b2 = load_const(bview("b2", 1, d), [1, d], DT.bfloat16, "b2")
    b2a = load_const(bview("b2a", 1, h2), [1, h2], DT.bfloat16, "b2a")
    lw_r0, _ = lay["lin_w"]
    lin_w16 = [load_const(blob[lw_r0 + k * 4:lw_r0 + (k + 1) * 4, :]
                          .rearrange("a b -> (a b)")
                          .rearrange("(p w) -> p w", p=P),
                          [P, 16], DT.bfloat16, f"lw{k}") for k in range(dt_)]
    lin_w = [t[:, 0:cfg.classes] for t in lin_w16]
    lin_bt = load_const(bview("lin_b", 1, 2 * cfg.classes), [1, 2 * cfg.classes],
                        DT.bfloat16, "lin_bt")
    lin_b = lin_bt[:].bitcast(DT.float32)
    iota_row = load_const(bview("iota_row", P, P), [P, P], DT.bfloat16,
                          "iota_row")
    iota_col = load_const(bview("iota_col", P, 1), [P, 1], DT.bfloat16,
                          "iota_col")
    drc = load_const(bview("drc", P, nb * tpb), [P, nb * tpb], DT.bfloat16,
                     "drc")
    bid_t = load_const(bview("bid", P, nb * 2), [P, nb * 2], DT.bfloat16,
                       "bid_t")
    wcol_t = load_const(bview("wcol", P, nb * 2), [P, nb * 2], DT.bfloat16,
                        "wcol_t")
    drow = bview("drow", 1, nb * epb)

    # replicate the 16 unique gather-idx rows to 128 partitions (bf16
    # container of int16 bits; bitcast at use)
    g_idx = pc.tile([P, nb * slot], DT.bfloat16, tag="g_idx")
    for k in range(8):
        nc.sync.dma_start(g_idx[k * 16:(k + 1) * 16, :],
                          bview("g16", 16, nb * slot))

    ones_bf = pc.tile([1, P], DT.bfloat16, tag="ones_bf")
    nc.vector.memset(ones_bf[:], 1.0)
    iota_colf = pc.tile([P, 1], DT.float32, tag="iota_colf")
    nc.vector.tensor_copy(iota_colf[:], iota_col[:])
    id_bf = pc.tile([P, P], DT.bfloat16, tag="id_bf")
    nc.vector.tensor_tensor(out=id_bf[:], in0=iota_row[:],
                            in1=iota_col[:].to_broadcast([P, P]), op=ALU.is_equal)
    id_f32 = pc.tile([P, P], DT.float32, tag="id_f32")
    nc.vector.tensor_copy(id_f32[:], id_bf[:])
    zrow_bf = pc.tile([1, tblw], DT.bfloat16, tag="zrow")
    nc.vector.memset(zrow_bf[:], 0.0)
    ones64_bf = pc.tile([1, cfg.g], DT.bfloat16, tag="ones64_bf")
    nc.vector.memset(ones64_bf[:], 1.0)
    lin_b_bf = pc.tile([1, cfg.classes], DT.bfloat16, tag="lin_b_bf")
    nc.vector.tensor_copy(lin_b_bf[:], lin_b)

    # a_dst per layer, kept in SBUF (bf16): [128, nb*h]
    adst_bf = pc.tile([P, nb * H], DT.bfloat16, tag="adst1")
    adst2_bf = pc.tile([P, nb * H], DT.bfloat16, tag="adst2")
    # h^T (bf16) for layer-2 matmuls: [128, dt_*nloc]
    hT = pc.tile([P, dt_ * nloc], DT.bfloat16, tag="hT")

    # DRAM tables
    loc_tbl1 = dram.tile([nloc, tblw], DT.bfloat16, tag="ltbl1")
    full_tbl1 = dram.tile([ntbl, tblw], DT.bfloat16, tag="ftbl1",
                          addr_space="Shared")
    loc_tbl2 = dram.tile([nloc, tblw], DT.bfloat16, tag="ltbl2")
    full_tbl2 = dram.tile([ntbl, tblw], DT.bfloat16, tag="ftbl2",
                          addr_space="Shared")

    # persistent pooling PSUM accumulator [128, dt_*G]
    ppool = ctx.enter_context(tc.tile_pool(name="ppool", bufs=1, space="PSUM"))
    p_pool = ppool.tile([P, dt_ * cfg.g], DT.float32, tag="p_pool")

    # ---------------- node phases ----------------
    def node_phase(layer):
        with tc.tile_pool(name=f"np{layer}", bufs=3) as sb, \
             tc.tile_pool(name=f"npp{layer}", bufs=2, space="PSUM") as ps:
            loc_tbl = loc_tbl1 if layer == 1 else loc_tbl2
            adst = adst_bf if layer == 1 else adst2_bf
            for k in range(nb):
                pxw = ps.tile([P, d], DT.float32, tag="pxw")
                pa = ps.tile([P, h2], DT.float32, tag="pa")
                if layer == 1:
                    lhs = [xt[c][:, k * P:(k + 1) * P] for c in range(ct)]
                    ws, was, bias, biasa = w1, w1a, b1, b1a
                else:
                    lhs = [hT[:, c * nloc + k * P:c * nloc + (k + 1) * P]
                           for c in range(dt_)]
                    ws, was, bias, biasa = w2, w2a, b2, b2a
                for c in range(len(lhs)):
                    nc.tensor.matmul(pxw[:], lhsT=lhs[c], rhs=ws[c][:],
                                     start=(c == 0), stop=False)
                    nc.tensor.matmul(pa[:], lhsT=lhs[c], rhs=was[c][:],
                                     start=(c == 0), stop=False)
                nc.tensor.matmul(pxw[:], lhsT=ones_bf[:], rhs=bias[:],
                                 start=False, stop=True)
                nc.tensor.matmul(pa[:], lhsT=ones_bf[:], rhs=biasa[:],
                                 start=False, stop=True)
                tbl = sb.tile([P, tblw], DT.bfloat16, tag="tbl")
                nc.scalar.copy(tbl[:, 0:d], pxw[:])
                nc.scalar.copy(tbl[:, d:d + H], pa[:, 0:H])
                nc.scalar.copy(tbl[:, d + H:tblw],
                               pa[:, 0:1].to_broadcast([P, tblw - d - H]))
                nc.vector.tensor_copy(adst[:, k * H:(k + 1) * H], pa[:, H:h2])
                nc.sync.dma_start(loc_tbl[k * P:(k + 1) * P, :], tbl[:])
            nc.sync.dma_start(loc_tbl[nloc - 1:nloc, :], zrow_bf[:])

    # ---------------- edge phases ----------------
    def edge_phase(layer):
        full_tbl = full_tbl1 if layer == 1 else full_tbl2
        adst = adst_bf if layer == 1 else adst2_bf
        ptr_bufs = 1 if layer == 1 else 2
        with tc.tile_pool(name=f"ep_g{layer}", bufs=2) as gp, \
             tc.tile_pool(name=f"ep_m{layer}", bufs=2) as mp, \
             tc.tile_pool(name=f"ep_s{layer}", bufs=2) as sb, \
             tc.tile_pool(name=f"ep_t{layer}", bufs=2) as tp, \
             tc.tile_pool(name=f"ep_pb{layer}", bufs=2, space="PSUM") as psb, \
             tc.tile_pool(name=f"ep_pa{layer}", bufs=2, space="PSUM") as psa, \
             tc.tile_pool(name=f"ep_po{layer}", bufs=2, space="PSUM") as pso:
            for b in range(nb):
                gath = gp.tile([P, tpb, tblw], DT.bfloat16, tag="gath")
                nc.gpsimd.dma_gather(
                    gath[:], full_tbl[:],
                    g_idx[:, b * slot:(b + 1) * slot].bitcast(DT.int16),
                    epb, epb, tblw, single_packet=False)
                bcr = gp.tile([1, epb], DT.bfloat16, tag="bcr")
                nc.sync.dma_start(bcr[:], drow[:, b * epb:(b + 1) * epb])

                # one-hot builds:
                #   m  [e, dst] from drc (batched is_equal)
                #   mt [dst, e] via PE row-broadcast of drow + is_equal
                m_t = mp.tile([P, epb], DT.bfloat16, tag="m_t")
                nc.vector.tensor_tensor(
                    out=m_t[:].rearrange("p (t c) -> p t c", t=tpb),
                    in0=iota_row[:].unsqueeze(1).to_broadcast([P, tpb, P]),
                    in1=drc[:, b * tpb:(b + 1) * tpb]
                        .unsqueeze(2).to_broadcast([P, tpb, P]),
                    op=ALU.is_equal)
                mt_t = mp.tile([P, epb], DT.bfloat16, tag="mt_t")
                pa_ps = psa.tile([P, tpb * H + H], DT.float32, tag="pa_ps")
                p_s = pa_ps[:, tpb * H:tpb * H + H]
                for g in range(ngrp):
                    t0 = g * GRP
                    gw = min(GRP, tpb - t0)
                    nw = gw * P
                    pbc = psb.tile([P, GRP * P], DT.float32, tag="pbc")
                    nc.tensor.matmul(pbc[:, 0:nw], lhsT=ones_bf[:],
                                     rhs=bcr[:, t0 * P:t0 * P + nw],
                                     start=True, stop=True)
                    pbcs = sb.tile([P, GRP * P], DT.bfloat16, tag="pbcs")
                    nc.scalar.copy(pbcs[:, 0:nw], pbc[:, 0:nw])
                    nc.vector.tensor_scalar(
                        out=mt_t[:, t0 * P:t0 * P + nw], in0=pbcs[:, 0:nw],
                        scalar1=iota_colf[:], scalar2=None, op0=ALU.is_equal)
                    for t in range(gw):
                        nc.tensor.matmul(
                            pa_ps[:, (t0 + t) * H:(t0 + t + 1) * H],
                            lhsT=mt_t[:, (t0 + t) * P:(t0 + t + 1) * P],
                            rhs=adst[:, b * H:(b + 1) * H],
                            start=True, stop=True)

                # e = lrelu(asrc + adst) ; ex = exp(e) (bf16)
                asrc_f = sb.tile([P, tpb * H], DT.float32, tag="asrc")
                nc.scalar.copy(
                    asrc_f[:].rearrange("p (a b) -> p a b", a=tpb),
                    gath[:, :, d:d + H])
                e_sum = sb.tile([P, tpb * H], DT.float32, tag="esum")
                nc.vector.tensor_tensor(out=e_sum[:], in0=asrc_f[:],
                                        in1=pa_ps[:, 0:tpb * H], op=ALU.add)
                e_lr = sb.tile([P, tpb * H], DT.float32, tag="elr")
                nc.vector.scalar_tensor_tensor(
                    out=e_lr[:], in0=e_sum[:], scalar=cfg.neg_slope,
                    in1=e_sum[:], op0=ALU.mult, op1=ALU.max)
                ex_b = sb.tile([P, tpb * H], DT.bfloat16, tag="exb")
                nc.scalar.activation(ex_b[:], e_lr[:], AF.Exp)

                # pass B: msg = feat * ex (DVE 2x, head-fast), segment sums
                p_out = pso.tile([P, d], DT.float32, tag="p_out")
                for g in range(ngrp):
                    t0 = g * GRP
                    gw = min(GRP, tpb - t0)
                    msg = mp.tile([P, GRP, d], DT.bfloat16, tag="msg")
                    nc.vector.tensor_tensor(
                        out=msg[:, 0:gw, :].rearrange("p t (c h) -> p t c h", h=H),
                        in0=gath[:, t0:t0 + gw, 0:d]
                            .rearrange("p t (c h) -> p t c h", h=H),
                        in1=ex_b[:, t0 * H:(t0 + gw) * H]
                            .rearrange("p (t h) -> p t h", t=gw)
                            .unsqueeze(2).to_broadcast([P, gw, cfg.hid, H]),
                        op=ALU.mult)
                    for t in range(gw):
                        tt = t0 + t
                        nc.tensor.matmul(p_s, lhsT=m_t[:, tt * P:(tt + 1) * P],
                                         rhs=ex_b[:, tt * H:(tt + 1) * H],
                                         start=(tt == 0), stop=(tt == tpb - 1))
                        nc.tensor.matmul(p_out[:], lhsT=m_t[:, tt * P:(tt + 1) * P],
                                         rhs=msg[:, t, :],
                                         start=(tt == 0), stop=(tt == tpb - 1))

                # normalize + elu
                s_g = sb.tile([P, H], DT.float32, tag="sg")
                nc.vector.tensor_scalar_max(s_g[:], p_s, 1e-30)
                rs = sb.tile([P, H], DT.float32, tag="rs")
                nc.vector.reciprocal(rs[:], s_g[:])
                outn = tp.tile([P, d], DT.float32, tag="outn")
                nc.vector.tensor_tensor(
                    out=outn[:].rearrange("p (a b) -> p a b", a=cfg.hid),
                    in0=p_out[:].rearrange("p (a b) -> p a b", a=cfg.hid),
                    in1=rs[:].unsqueeze(1).to_broadcast([P, cfg.hid, H]),
                    op=ALU.mult)
                mn = tp.tile([P, d], DT.float32, tag="mn")
                nc.vector.tensor_scalar_min(mn[:], outn[:], 0.0)
                ee = tp.tile([P, d], DT.float32, tag="ee")
                nc.scalar.activation(ee[:], mn[:], AF.Exp)
                h_f = tp.tile([P, d], DT.float32, tag="hf")
                nc.vector.scalar_tensor_tensor(
                    out=h_f[:], in0=ee[:], scalar=-1.0, in1=outn[:],
                    op0=ALU.add, op1=ALU.max)
                h_b = tp.tile([P, d], DT.bfloat16, tag="hb")
                nc.vector.tensor_copy(h_b[:], h_f[:])

                if layer == 1:
                    for c in range(dt_):
                        ptr = psb.tile([P, P], DT.bfloat16, tag="ptr",
                                       bufs=ptr_bufs)
                        nc.tensor.transpose(ptr[:], h_b[:, c * P:(c + 1) * P],
                                            id_bf[:])
                        nc.scalar.copy(
                            hT[:, c * nloc + b * P:c * nloc + (b + 1) * P],
                            ptr[:])
                else:
                    mbb = tp.tile([P, cfg.g], DT.bfloat16, tag="mbb")
                    nc.vector.tensor_scalar(
                        out=mbb[:], in0=iota_row[:, 0:cfg.g],
                        scalar1=bid_t[:, 2 * b:2 * b + 2].bitcast(DT.float32),
                        scalar2=wcol_t[:, 2 * b:2 * b + 2].bitcast(DT.float32),
                        op0=ALU.is_equal, op1=ALU.mult)
                    for c in range(dt_):
                        nc.tensor.matmul(
                            p_pool[:, c * cfg.g:(c + 1) * cfg.g],
                            lhsT=h_b[:, c * P:(c + 1) * P],
                            rhs=mbb[:],
                            start=(b == 0 and c == 0),
                            stop=(b == nb - 1 and c == dt_ - 1))

    def gather_table(loc, full):
        if cfg.ncore == 1 or "nocc" in ablate:
            nc.sync.dma_start(full[:cfg.nloc, :], loc[:])
        else:
            nc.gpsimd.collective_compute(
                "AllGather", ALU.bypass, replica_groups=groups,
                ins=[loc[:].opt()], outs=[full[:].opt()])

    # ---------------- run phases ----------------
    if "nonp" not in ablate:
        node_phase(1)
    gather_table(loc_tbl1, full_tbl1)
    if "noedge" not in ablate:
        edge_phase(1)
    else:
        with tc.tile_pool(name="abl", bufs=1) as ab:
            nc.vector.memset(hT[:, 0:P], 0.0)
            pzf = ab.tile([P, P], DT.float32, tag="pzf")
            nc.vector.memset(pzf[:], 0.0)
            zr64 = ab.tile([P, cfg.g], DT.float32, tag="zr64")
            nc.vector.memset(zr64[:], 0.0)
            for c in range(dt_):
                nc.tensor.matmul(p_pool[:, c * cfg.g:(c + 1) * cfg.g],
                                 lhsT=pzf[:], rhs=zr64[:],
                                 start=(c == 0), stop=(c == dt_ - 1))
    if "nonp" not in ablate:
        node_phase(2)
    gather_table(loc_tbl2, full_tbl2)
    if "noedge" not in ablate:
        edge_phase(2)

    # ---------------- pooling reduce + classifier ----------------
    with tc.tile_pool(name="fin", bufs=1) as sb, \
         tc.tile_pool(name="finp", bufs=1, space="PSUM") as ps:
        pool_sb = sb.tile([P, dt_ * cfg.g], DT.float32, tag="pool_sb")
        nc.vector.tensor_copy(pool_sb[:], p_pool[:])
        pool_g0 = sb.tile([P, dt_ * cfg.g], DT.float32, tag="pool_g0")
        if cfg.ncore == 1 or "nocc" in ablate:
            nc.vector.tensor_copy(pool_g0[:], pool_sb[:])
        else:
            pool_l = dram.tile([P, dt_ * cfg.g], DT.float32, tag="pool_l")
            pool_r = dram.tile([P, dt_ * cfg.g], DT.float32, tag="pool_r")
            nc.sync.dma_start(pool_l[:], pool_sb[:])
            nc.gpsimd.collective_compute(
                "AllReduce", ALU.add, replica_groups=groups,
                ins=[pool_l[:].opt()], outs=[pool_r[:].opt()])
            nc.sync.dma_start(pool_g0[:], pool_r[:])
        pool_gb = sb.tile([P, dt_ * cfg.g], DT.bfloat16, tag="pool_gb")
        nc.vector.tensor_copy(pool_gb[:], pool_g0[:])

        p_lg = ps.tile([cfg.classes, cfg.g], DT.float32, tag="p_lg")
        for c in range(dt_):
            nc.tensor.matmul(p_lg[:], lhsT=lin_w[c],
                             rhs=pool_gb[:, c * cfg.g:(c + 1) * cfg.g],
                             start=(c == 0), stop=False)
        nc.tensor.matmul(p_lg[:], lhsT=lin_b_bf[:], rhs=ones64_bf[:],
                         start=False, stop=True)
        lg_sb = sb.tile([cfg.classes, cfg.g], DT.float32, tag="lg_sb")
        nc.vector.tensor_copy(lg_sb[:], p_lg[:])
        p_t = ps.tile([cfg.g, cfg.classes], DT.float32, tag="p_t")
        nc.tensor.transpose(p_t[:], lg_sb[:], id_f32[:cfg.classes, :cfg.classes])
        logit = sb.tile([cfg.g, cfg.classes], DT.float32, tag="logit")
        nc.vector.tensor_copy(logit[:], p_t[:])

        rmax = sb.tile([cfg.g, 1], DT.float32, tag="rmax")
        nc.vector.reduce_max(rmax[:], logit[:], axis=mybir.AxisListType.X)
        sh = sb.tile([cfg.g, cfg.classes], DT.float32, tag="sh")
        nc.vector.tensor_scalar(out=sh[:], in0=logit[:], scalar1=rmax[:],
                                scalar2=None, op0=ALU.subtract)
        exps = sb.tile([cfg.g, cfg.classes], DT.float32, tag="exps")
        nc.scalar.activation(exps[:], sh[:], AF.Exp)
        ssum = sb.tile([cfg.g, 1], DT.float32, tag="ssum")
        nc.vector.reduce_sum(ssum[:], exps[:], axis=mybir.AxisListType.X)
        lns = sb.tile([cfg.g, 1], DT.float32, tag="lns")
        nc.scalar.activation(lns[:], ssum[:], AF.Ln)
        res = sb.tile([cfg.g, cfg.classes], DT.float32, tag="res")
        nc.vector.tensor_scalar(out=res[:], in0=sh[:], scalar1=lns[:],
                                scalar2=None, op0=ALU.subtract)
        nc.sync.dma_start(o_out[:], res[:])


# --------------------------------------------------------------------------
# Program build + run
# --------------------------------------------------------------------------

def build_program(cfg: GATConfig, tpb: int):
    from concourse import bacc
    nc = bacc.Bacc("TRN2", target_bir_lowering=False, debug=False,
                   num_devices=cfg.ncore)
    nb, nloc, h2 = cfg.nb, cfg.nloc, 2 * cfg.heads
    epb = tpb * P
    ins = {}

    def inp(name, shape, dt):
        ins[name] = nc.dram_tensor(name, list(shape), dt, kind="ExternalInput").ap()

    _, R = blob_layout(cfg, tpb)
    inp("blob", [R, 512], DT.bfloat16)

    out_ap = nc.dram_tensor("out", [cfg.g, cfg.classes], DT.float32,
                            kind="ExternalOutput").ap()

    with tile.TileContext(nc) as tc:
        gat_tile_kernel(tc, cfg, tpb, [out_ap], ins)
    nc.compile()
    return nc


_CACHE = {}


def _prepare(cfg: GATConfig, inputs):
    import hashlib
    edge_index = np.asarray(inputs["edge_index"])
    batch = np.asarray(inputs["batch"])
    key = hashlib.sha1(edge_index.tobytes() + batch.tobytes()).hexdigest()
    if key in _CACHE:
        return _CACHE[key]
    tpb, cores, consts = build_host_data(cfg, edge_index, batch)
    nc = build_program(cfg, tpb)
    _CACHE[key] = (nc, tpb, cores, consts)
    return _CACHE[key]


def make_in_maps(cfg: GATConfig, inputs, cores, consts):
    wd = build_weight_data(cfg, inputs["W1"], inputs["att_src1"], inputs["att_dst1"],
                           inputs["bias1"], inputs["W2"], inputs["att_src2"],
                           inputs["att_dst2"], inputs["bias2"], inputs["lin_w"],
                           inputs["lin_b"])
    x = np.asarray(inputs["x"], dtype=np.float32)
    x_t_full = np.ascontiguousarray(x.T).astype(BF16)   # [in_dim, n]
    node_at = consts["node_at"]
    wshard = WBR // cfg.ncore
    tpb = consts["tpb"]
    lay, R = blob_layout(cfg, tpb)

    def put(blob, name, arr_bf16_flat):
        r0, rows = lay[name]
        flat = np.ascontiguousarray(arr_bf16_flat).reshape(-1)
        assert flat.size <= rows * 512, (name, flat.size, rows)
        blob.reshape(-1)[r0 * 512:r0 * 512 + flat.size] = flat

    lwp = np.zeros((cfg.d, 16), dtype=BF16)
    lwp[:, :cfg.classes] = wd["lin_w"]
    iota_row = np.tile(np.arange(P, dtype=np.float32).reshape(1, P),
                       (P, 1)).astype(BF16)
    iota_col = np.arange(P, dtype=np.float32).astype(BF16)

    in_maps = []
    for c in range(cfg.ncore):
        xt = np.zeros((cfg.in_dim, cfg.nloc), dtype=BF16)
        nodes = node_at[c]
        real = nodes >= 0
        xt[:, real] = x_t_full[:, nodes[real]]
        blob = np.zeros((R, 512), dtype=BF16)
        put(blob, "x_t", xt)
        put(blob, "wsh", wd["wblob"][c * wshard:(c + 1) * wshard])
        put(blob, "w1a", wd["w1a"])
        put(blob, "w2a", wd["w2a"])
        put(blob, "b1", wd["b1"])
        put(blob, "b2", wd["b2"])
        put(blob, "b1a", wd["b1a"])
        put(blob, "b2a", wd["b2a"])
        put(blob, "lin_w", lwp)
        put(blob, "iota_row", iota_row)
        put(blob, "iota_col", iota_col)
        put(blob, "drc", cores[c]["drc"])
        put(blob, "drow", cores[c]["drow"])
        put(blob, "g16", cores[c]["g16"].view(BF16))
        put(blob, "bid", cores[c]["bid"].view(BF16))
        put(blob, "wcol", cores[c]["wcol"].view(BF16))
        put(blob, "lin_b", wd["lin_b"].view(BF16))
        in_maps.append(dict(blob=blob))
    return in_maps


def run(cfg: GATConfig, inputs, trace=False):
    from concourse.bass_utils import run_bass_kernel_spmd
    nc, tpb, cores, consts = _prepare(cfg, inputs)
    in_maps = make_in_maps(cfg, inputs, cores, consts)
    res = run_bass_kernel_spmd(nc, in_maps, core_ids=list(range(cfg.ncore)),
                               trace=trace)
    return res


def kernel(**inputs) -> np.ndarray:
    res = run(CFG, inputs, trace=False)
    return np.asarray(res.results[0]["out"])
